# revision 2
# baseline (speedup 1.0000x reference)
"""Bass/Trainium2 kernel for additive (Bahdanau-style) multi-head attention.

Reference computation (B=2, S=512, D=512, H=8, HD=64):
    q = heads(query @ Wq + bq); k = heads(key_ @ Wk + bk); v = heads(value @ Wv + bv)
    scores[b,h,i,j] = sum_d tanh((q @ Aq)[b,h,i,d] + (k @ Ak)[b,h,j,d]) * av[d]
    attn = softmax(scores, -1); ctx = attn @ v; out = merge(ctx) @ Wo + bo
    returns (out, attn)

Sharding: 8 cores; core c handles batch b = c // 4 and head pair
h0 = 2*(c % 4), h0+1.  Each core computes its two heads' attention and a
partial output projection; the host sums the 4 partial outputs per batch.

Per-core device plan:
  - Projections on TensorE (fp32, exact): qT2/kT2 [128=(h,hd), 512] then
    block-diag Aq/Ak matmul -> qAT2/kAT2 [128, 512(tokens)].
  - For each query q: ScalarE does tanh(kAT2 + qAT2[:, q]) in one ACTIVATE
    (per-partition bias), output rounded to float32r.
  - TensorE reduces over d with the av vector at full speed (float32r,
    1 cyc/row): lhsT is a shifted view into a [128, 256] buffer whose columns
    128/129 hold av for head0/head1 rows, so query q accumulates into PSUM
    rows (2*(q%64), 2*(q%64)+1) of a [128, 512] score bank; 64 queries fill
    one bank (rows interleave (query, head)).
  - Softmax per row: DVE max (negated), ScalarE Exp(bias=-max, accum_out=sums)
    reading PSUM directly, DVE reciprocal + per-partition multiply.
  - attn rows DMA out interleaved; host de-interleaves.
  - PE transpose of attn tiles -> attnT [k, (group,query,head)] for the
    context matmul; ctx^T [(h,hd), q] accumulated in PSUM (+bv per partition,
    exact because softmax rows sum to 1); output projection vs Wo row-slice
    (+bo/4 so the host-side sum of 4 partials reconstructs bo).
"""

import numpy as np

import concourse.bass as bass
import concourse.mybir as mybir
import concourse.tile as tile
from concourse import bacc
from concourse.bass_utils import run_bass_kernel_spmd
from concourse.masks import make_identity

F32 = mybir.dt.float32
F32R = mybir.dt.float32r
AF = mybir.ActivationFunctionType

B, S, D, H = 2, 512, 512, 8
HD = D // H  # 64
HP = 2  # heads per core
NCORES = 8
G = S // 64  # score groups of 64 queries -> 8


def build_nc():
    nc = bacc.Bacc("TRN2", target_bir_lowering=False, debug=False, num_devices=NCORES)

    # ---- DRAM I/O (per-core shards; same names on every core) ----
    xqT = nc.dram_tensor("xqT", [D, S], F32, kind="ExternalInput")
    xkT = nc.dram_tensor("xkT", [D, S], F32, kind="ExternalInput")
    xvT = nc.dram_tensor("xvT", [D, S], F32, kind="ExternalInput")
    wq2 = nc.dram_tensor("wq2", [D, 128], F32, kind="ExternalInput")
    wk2 = nc.dram_tensor("wk2", [D, 128], F32, kind="ExternalInput")
    wv2 = nc.dram_tensor("wv2", [D, 128], F32, kind="ExternalInput")
    wo2 = nc.dram_tensor("wo2", [128, D], F32, kind="ExternalInput")
    bq2 = nc.dram_tensor("bq2", [128, 1], F32, kind="ExternalInput")
    bk2 = nc.dram_tensor("bk2", [128, 1], F32, kind="ExternalInput")
    bv2 = nc.dram_tensor("bv2", [128, 1], F32, kind="ExternalInput")
    bo4 = nc.dram_tensor("bo4", [1, D], F32, kind="ExternalInput")
    aq = nc.dram_tensor("aq", [HD, HD], F32, kind="ExternalInput")
    ak = nc.dram_tensor("ak", [HD, HD], F32, kind="ExternalInput")
    av2 = nc.dram_tensor("av2", [HD, 1], F32, kind="ExternalInput")
    attn_out = nc.dram_tensor("attn_out", [2 * S, S], F32, kind="ExternalOutput")
    out_part = nc.dram_tensor("out_part", [S, D], F32, kind="ExternalOutput")

    with tile.TileContext(nc) as tc:
        with (
            tc.tile_pool(name="const", bufs=1) as const,
            tc.tile_pool(name="tpool", bufs=3) as tpool,
            tc.tile_pool(name="apool", bufs=2) as apool,
            tc.tile_pool(name="stats", bufs=4) as stats,
            tc.tile_pool(name="ps_sc", bufs=2, space="PSUM") as ps_sc,
            tc.tile_pool(name="ps_tp", bufs=2, space="PSUM") as ps_tp,
            tc.tile_pool(name="ps_mm", bufs=2, space="PSUM") as ps_mm,
        ):
            # ---------- load inputs ----------
            xq_t = []
            xk_t = []
            xv_t = []
            for m in range(4):
                t = const.tile([128, S], F32, name=f"xq_{m}")
                nc.sync.dma_start(t[:], xqT[m * 128 : (m + 1) * 128, :])
                xq_t.append(t)
                t = const.tile([128, S], F32, name=f"xk_{m}")
                nc.sync.dma_start(t[:], xkT[m * 128 : (m + 1) * 128, :])
                xk_t.append(t)
                t = const.tile([128, S], F32, name=f"xv_{m}")
                nc.sync.dma_start(t[:], xvT[m * 128 : (m + 1) * 128, :])
                xv_t.append(t)
            wq_t = []
            wk_t = []
            wv_t = []
            for m in range(4):
                t = const.tile([128, 128], F32, name=f"wq_{m}")
                nc.sync.dma_start(t[:], wq2[m * 128 : (m + 1) * 128, :])
                wq_t.append(t)
                t = const.tile([128, 128], F32, name=f"wk_{m}")
                nc.sync.dma_start(t[:], wk2[m * 128 : (m + 1) * 128, :])
                wk_t.append(t)
                t = const.tile([128, 128], F32, name=f"wv_{m}")
                nc.sync.dma_start(t[:], wv2[m * 128 : (m + 1) * 128, :])
                wv_t.append(t)
            wo_t = const.tile([128, D], F32, name="wo_t")
            nc.sync.dma_start(wo_t[:], wo2[:, :])
            bq_t = const.tile([128, 1], F32, name="bq_t")
            nc.sync.dma_start(bq_t[:], bq2[:, :])
            bk_t = const.tile([128, 1], F32, name="bk_t")
            nc.sync.dma_start(bk_t[:], bk2[:, :])
            bv_t = const.tile([128, 1], F32, name="bv_t")
            nc.sync.dma_start(bv_t[:], bv2[:, :])
            # bo/4 replicated across partitions via stride-0 DMA
            bo_rep = const.tile([128, D], F32, name="bo_rep")
            bo_bcast = bass.AP(tensor=bo4.ap().tensor, offset=0, ap=[[0, 128], [1, D]])
            nc.sync.dma_start(bo_rep[:], bo_bcast)

            # block-diagonal Aq/Ak [128, 128]
            aq2 = const.tile([128, 128], F32, name="aq2")
            nc.vector.memset(aq2[:], 0.0)
            nc.sync.dma_start(aq2[0:HD, 0:HD], aq[:, :])
            nc.sync.dma_start(aq2[HD:128, HD:128], aq[:, :])
            ak2 = const.tile([128, 128], F32, name="ak2")
            nc.vector.memset(ak2[:], 0.0)
            nc.sync.dma_start(ak2[0:HD, 0:HD], ak[:, :])
            nc.sync.dma_start(ak2[HD:128, HD:128], ak[:, :])

            # shifted-av buffer (f32r): col 128 = av on head0 rows, col 129 on head1
            av_sb = const.tile([128, 1], F32, name="av_sb")
            nc.sync.dma_start(av_sb[0:HD, :], av2[:, :])
            nc.sync.dma_start(av_sb[HD:128, :], av2[:, :])
            zz = const.tile([128, 256], F32, name="zz")
            nc.vector.memset(zz[:], 0.0)
            avb = const.tile([128, 256], F32R, name="avb")
            nc.vector.tensor_copy(avb[:], zz[:])
            nc.vector.tensor_copy(avb[0:HD, 128:129], av_sb[0:HD, :])
            nc.vector.tensor_copy(avb[HD:128, 129:130], av_sb[HD:128, :])

            ident = const.tile([128, 128], F32, name="ident")
            make_identity(nc, ident[:])

            # ---------- projections ----------
            # qT2 = (Xq @ Wq2)^T + bq2 ; qAT2 = blockdiag(Aq)^T-matmul
            qat2 = const.tile([128, S], F32, name="qat2")
            kat2 = const.tile([128, S], F32, name="kat2")
            for x_t, w_t, b_t, a2, outT in (
                (xq_t, wq_t, bq_t, aq2, qat2),
                (xk_t, wk_t, bk_t, ak2, kat2),
            ):
                pp = ps_mm.tile([128, S], F32, tag="mm", name="pp")
                for m in range(4):
                    nc.tensor.matmul(
                        pp[:], w_t[m][:], x_t[m][:], start=(m == 0), stop=(m == 3)
                    )
                pb = const.tile([128, S], F32, name="pb")
                nc.vector.tensor_scalar_add(pb[:], pp[:], b_t[:])
                pa = ps_mm.tile([128, S], F32, tag="mm", name="pa")
                nc.tensor.matmul(pa[:], a2[:], pb[:], start=True, stop=True)
                nc.vector.tensor_copy(outT[:], pa[:])

            # v [k, (h,hd)] as 4 tiles of [128, 128] (no bias; bv folded later)
            v_t = []
            for kc in range(4):
                pv = ps_mm.tile([128, S], F32, tag="mm", name="pv")
                for m in range(4):
                    nc.tensor.matmul(
                        pv[:, 0:128],
                        xv_t[m][:, kc * 128 : (kc + 1) * 128],
                        wv_t[m][:],
                        start=(m == 0),
                        stop=(m == 3),
                    )
                vt = const.tile([128, 128], F32, name=f"v_{kc}")
                nc.vector.tensor_copy(vt[:], pv[:, 0:128])
                v_t.append(vt)

            # attnT chunks: [k-chunk partitions, (group, query, head) columns]
            attnT = [
                const.tile([128, 2 * S], F32, name=f"attnT_{c}") for c in range(4)
            ]

            # ---------- main loop: scores + softmax + transpose ----------
            for g in range(G):
                sc_ps = ps_sc.tile([128, S], F32, tag="sc", name="sc_ps")
                for i in range(64):
                    q = g * 64 + i
                    tt = tpool.tile([128, S], F32R, tag="T", name="tt")
                    nc.scalar.activation(
                        tt[:], kat2[:], AF.Tanh, bias=qat2[:, q : q + 1]
                    )
                    nc.tensor.matmul(
                        sc_ps[:],
                        avb[:, 128 - 2 * i : 256 - 2 * i],
                        tt[:],
                        start=(i == 0),
                        stop=(i == 63),
                    )
                # softmax over the 512 free-dim entries of each (q, h) row
                mx = stats.tile([128, 1], F32, tag="mx", name="mx")
                nc.vector.tensor_reduce(
                    mx[:], sc_ps[:], axis=mybir.AxisListType.X,
                    op=mybir.AluOpType.max, negate=True,
                )
                esum = stats.tile([128, 1], F32, tag="esum", name="esum")
                attn_e = apool.tile([128, S], F32, tag="attn_e", name="attn_e")
                nc.scalar.activation(
                    attn_e[:], sc_ps[:], AF.Exp, bias=mx[:], accum_out=esum[:]
                )
                rec = stats.tile([128, 1], F32, tag="rec", name="rec")
                nc.vector.reciprocal(rec[:], esum[:])
                attn_n = apool.tile([128, S], F32, tag="attn_n", name="attn_n")
                nc.vector.tensor_scalar_mul(attn_n[:], attn_e[:], rec[:])
                nc.sync.dma_start(attn_out[g * 128 : (g + 1) * 128, :], attn_n[:])
                for c in range(4):
                    tp = ps_tp.tile([128, 128], F32, tag="tp", name="tp")
                    nc.tensor.transpose(
                        tp[:], attn_n[:, c * 128 : (c + 1) * 128], ident[:]
                    )
                    nc.vector.tensor_copy(
                        attnT[c][:, g * 128 : (g + 1) * 128], tp[:]
                    )

            # ---------- context: ctx^T[(h,hd), q] ----------
            ctx_ps = ps_mm.tile([128, S], F32, tag="mm", name="ctx_ps")
            for h in range(HP):
                for c in range(4):
                    rhs = attnT[c][:].rearrange(
                        "p (g i h) -> p g i h", g=G, i=64, h=HP
                    )[:, :, :, h]
                    nc.tensor.matmul(
                        ctx_ps[h * HD : (h + 1) * HD, :],
                        v_t[c][:, h * HD : (h + 1) * HD],
                        rhs,
                        start=(c == 0),
                        stop=(c == 3),
                        skip_group_check=True,
                    )
            ctxT = const.tile([128, S], F32, name="ctxT")
            nc.vector.tensor_scalar_add(ctxT[:], ctx_ps[:], bv_t[:])

            # ---------- output projection ----------
            for sc in range(4):
                op_ps = ps_mm.tile([128, S], F32, tag="mm", name="op_ps")
                nc.tensor.matmul(
                    op_ps[:], ctxT[:, sc * 128 : (sc + 1) * 128], wo_t[:],
                    start=True, stop=True,
                )
                ob = apool.tile([128, S], F32, tag="ob", name="ob")
                nc.vector.tensor_add(ob[:], op_ps[:], bo_rep[:])
                nc.sync.dma_start(out_part[sc * 128 : (sc + 1) * 128, :], ob[:])

    nc.compile()
    return nc


_NC_CACHE = None


def _get_nc():
    global _NC_CACHE
    if _NC_CACHE is None:
        _NC_CACHE = build_nc()
    return _NC_CACHE


def _prep_core_inputs(c, query, key_, value, Wq, bq, Wk, bk, Wv, bv, Wo, bo, Aq, Ak, av):
    b = c // 4
    hp = c % 4
    cols = slice(hp * 128, hp * 128 + 128)
    cc = np.ascontiguousarray
    return {
        "xqT": cc(query[b].T),
        "xkT": cc(key_[b].T),
        "xvT": cc(value[b].T),
        "wq2": cc(Wq[:, cols]),
        "wk2": cc(Wk[:, cols]),
        "wv2": cc(Wv[:, cols]),
        "wo2": cc(Wo[cols, :]),
        "bq2": cc(bq[cols][:, None]),
        "bk2": cc(bk[cols][:, None]),
        "bv2": cc(bv[cols][:, None]),
        "bo4": cc((bo * 0.25)[None, :]),
        "aq": cc(Aq),
        "ak": cc(Ak),
        "av2": cc(av[:, None]),
    }


def kernel(**inputs):
    f = lambda name: np.asarray(inputs[name], dtype=np.float32)
    args = (
        f("query"), f("key_"), f("value"),
        f("Wq"), f("bq"), f("Wk"), f("bk"), f("Wv"), f("bv"),
        f("Wo"), f("bo"), f("Aq"), f("Ak"), f("av"),
    )
    nc = _get_nc()
    in_maps = [_prep_core_inputs(c, *args) for c in range(NCORES)]
    res = run_bass_kernel_spmd(nc, in_maps, core_ids=list(range(NCORES)))
    results = res.results

    attn = np.empty((B, H, S, S), dtype=np.float32)
    out = np.zeros((B, S, D), dtype=np.float32)
    for c in range(NCORES):
        b = c // 4
        hp = c % 4
        a = results[c]["attn_out"]  # [1024, 512] rows = (g, i, h) interleaved
        a = a.reshape(G, 64, HP, S).transpose(2, 0, 1, 3).reshape(HP, S, S)
        attn[b, 2 * hp : 2 * hp + 2] = a
        out[b] += results[c]["out_part"]
    return out, attn


# revision 3
# speedup vs baseline: 1.0084x; 1.0084x over previous
"""Bass/Trainium2 kernel for additive (Bahdanau-style) multi-head attention.

Reference computation (B=2, S=512, D=512, H=8, HD=64):
    q = heads(query @ Wq + bq); k = heads(key_ @ Wk + bk); v = heads(value @ Wv + bv)
    scores[b,h,i,j] = sum_d tanh((q @ Aq)[b,h,i,d] + (k @ Ak)[b,h,j,d]) * av[d]
    attn = softmax(scores, -1); ctx = attn @ v; out = merge(ctx) @ Wo + bo
    returns (out, attn)

Sharding: 8 cores; core c handles batch b = c // 4 and head pair
h0 = 2*(c % 4), h0+1.  Each core computes its two heads' attention and a
partial output projection; the host sums the 4 partial outputs per batch.

Per-core device plan:
  - Projections on TensorE (fp32, exact): qT2/kT2 [128=(h,hd), 512] then
    block-diag Aq/Ak matmul -> qAT2/kAT2 [128, 512(tokens)].
  - For each query q: ScalarE does tanh(kAT2 + qAT2[:, q]) in one ACTIVATE
    (per-partition bias), output rounded to float32r.
  - TensorE reduces over d with the av vector at full speed (float32r,
    1 cyc/row): lhsT is a shifted view into a [128, 256] buffer whose columns
    128/129 hold av for head0/head1 rows, so query q accumulates into PSUM
    rows (2*(q%64), 2*(q%64)+1) of a [128, 512] score bank; 64 queries fill
    one bank (rows interleave (query, head)).
  - Softmax per row: DVE max (negated), ScalarE Exp(bias=-max, accum_out=sums)
    reading PSUM directly, DVE reciprocal + per-partition multiply.
  - attn rows DMA out interleaved; host de-interleaves.
  - PE transpose of attn tiles -> attnT [k, (group,query,head)] for the
    context matmul; ctx^T [(h,hd), q] accumulated in PSUM (+bv per partition,
    exact because softmax rows sum to 1); output projection vs Wo row-slice
    (+bo/4 so the host-side sum of 4 partials reconstructs bo).
"""

import numpy as np

import concourse.bass as bass
import concourse.mybir as mybir
import concourse.tile as tile
from concourse import bacc
from concourse.bass_utils import run_bass_kernel_spmd
from concourse.masks import make_identity

F32 = mybir.dt.float32
F32R = mybir.dt.float32r
BF16 = mybir.dt.bfloat16
RED_DT = BF16  # dtype of tanh output + av weights for the d-reduction matmul
AF = mybir.ActivationFunctionType

B, S, D, H = 2, 512, 512, 8
HD = D // H  # 64
HP = 2  # heads per core
NCORES = 8
G = S // 64  # score groups of 64 queries -> 8


def build_nc():
    nc = bacc.Bacc("TRN2", target_bir_lowering=False, debug=False, num_devices=NCORES)

    # ---- DRAM I/O (per-core shards; same names on every core) ----
    xqT = nc.dram_tensor("xqT", [D, S], F32, kind="ExternalInput")
    xkT = nc.dram_tensor("xkT", [D, S], F32, kind="ExternalInput")
    xvT = nc.dram_tensor("xvT", [D, S], F32, kind="ExternalInput")
    wq2 = nc.dram_tensor("wq2", [D, 128], F32, kind="ExternalInput")
    wk2 = nc.dram_tensor("wk2", [D, 128], F32, kind="ExternalInput")
    wv2 = nc.dram_tensor("wv2", [D, 128], F32, kind="ExternalInput")
    wo2 = nc.dram_tensor("wo2", [128, D], F32, kind="ExternalInput")
    bq2 = nc.dram_tensor("bq2", [128, 1], F32, kind="ExternalInput")
    bk2 = nc.dram_tensor("bk2", [128, 1], F32, kind="ExternalInput")
    bv2 = nc.dram_tensor("bv2", [128, 1], F32, kind="ExternalInput")
    bo4 = nc.dram_tensor("bo4", [1, D], F32, kind="ExternalInput")
    aq = nc.dram_tensor("aq", [HD, HD], F32, kind="ExternalInput")
    ak = nc.dram_tensor("ak", [HD, HD], F32, kind="ExternalInput")
    av2 = nc.dram_tensor("av2", [HD, 1], F32, kind="ExternalInput")
    attn_out = nc.dram_tensor("attn_out", [2 * S, S], F32, kind="ExternalOutput")
    out_part = nc.dram_tensor("out_part", [S, D], F32, kind="ExternalOutput")

    with tile.TileContext(nc) as tc:
        with (
            tc.tile_pool(name="const", bufs=1) as const,
            tc.tile_pool(name="tpool", bufs=3) as tpool,
            tc.tile_pool(name="apool", bufs=2) as apool,
            tc.tile_pool(name="stats", bufs=4) as stats,
            tc.tile_pool(name="ps_sc", bufs=2, space="PSUM") as ps_sc,
            tc.tile_pool(name="ps_tp", bufs=2, space="PSUM") as ps_tp,
            tc.tile_pool(name="ps_mm", bufs=2, space="PSUM") as ps_mm,
        ):
            # ---------- load inputs ----------
            xq_t = []
            xk_t = []
            xv_t = []
            for m in range(4):
                t = const.tile([128, S], F32, name=f"xq_{m}")
                nc.sync.dma_start(t[:], xqT[m * 128 : (m + 1) * 128, :])
                xq_t.append(t)
                t = const.tile([128, S], F32, name=f"xk_{m}")
                nc.sync.dma_start(t[:], xkT[m * 128 : (m + 1) * 128, :])
                xk_t.append(t)
                t = const.tile([128, S], F32, name=f"xv_{m}")
                nc.sync.dma_start(t[:], xvT[m * 128 : (m + 1) * 128, :])
                xv_t.append(t)
            wq_t = []
            wk_t = []
            wv_t = []
            for m in range(4):
                t = const.tile([128, 128], F32, name=f"wq_{m}")
                nc.sync.dma_start(t[:], wq2[m * 128 : (m + 1) * 128, :])
                wq_t.append(t)
                t = const.tile([128, 128], F32, name=f"wk_{m}")
                nc.sync.dma_start(t[:], wk2[m * 128 : (m + 1) * 128, :])
                wk_t.append(t)
                t = const.tile([128, 128], F32, name=f"wv_{m}")
                nc.sync.dma_start(t[:], wv2[m * 128 : (m + 1) * 128, :])
                wv_t.append(t)
            wo_t = const.tile([128, D], F32, name="wo_t")
            nc.sync.dma_start(wo_t[:], wo2[:, :])
            bq_t = const.tile([128, 1], F32, name="bq_t")
            nc.sync.dma_start(bq_t[:], bq2[:, :])
            bk_t = const.tile([128, 1], F32, name="bk_t")
            nc.sync.dma_start(bk_t[:], bk2[:, :])
            bv_t = const.tile([128, 1], F32, name="bv_t")
            nc.sync.dma_start(bv_t[:], bv2[:, :])
            # bo/4 replicated across partitions via stride-0 DMA
            bo_rep = const.tile([128, D], F32, name="bo_rep")
            bo_bcast = bass.AP(tensor=bo4.ap().tensor, offset=0, ap=[[0, 128], [1, D]])
            nc.sync.dma_start(bo_rep[:], bo_bcast)

            # block-diagonal Aq/Ak [128, 128]
            aq2 = const.tile([128, 128], F32, name="aq2")
            nc.vector.memset(aq2[:], 0.0)
            nc.sync.dma_start(aq2[0:HD, 0:HD], aq[:, :])
            nc.sync.dma_start(aq2[HD:128, HD:128], aq[:, :])
            ak2 = const.tile([128, 128], F32, name="ak2")
            nc.vector.memset(ak2[:], 0.0)
            nc.sync.dma_start(ak2[0:HD, 0:HD], ak[:, :])
            nc.sync.dma_start(ak2[HD:128, HD:128], ak[:, :])

            # shifted-av buffer (f32r): col 128 = av on head0 rows, col 129 on head1
            av_sb = const.tile([128, 1], F32, name="av_sb")
            nc.sync.dma_start(av_sb[0:HD, :], av2[:, :])
            nc.sync.dma_start(av_sb[HD:128, :], av2[:, :])
            avb = const.tile([128, 256], RED_DT, name="avb")
            nc.vector.memset(avb[:], 0.0)
            nc.vector.tensor_copy(avb[0:HD, 128:129], av_sb[0:HD, :])
            nc.vector.tensor_copy(avb[HD:128, 129:130], av_sb[HD:128, :])

            ident = const.tile([128, 128], F32, name="ident")
            make_identity(nc, ident[:])

            # ---------- projections ----------
            # qT2 = (Xq @ Wq2)^T + bq2 ; qAT2 = blockdiag(Aq)^T-matmul
            qat2 = const.tile([128, S], F32, name="qat2")
            kat2 = const.tile([128, S], F32, name="kat2")
            for x_t, w_t, b_t, a2, outT in (
                (xq_t, wq_t, bq_t, aq2, qat2),
                (xk_t, wk_t, bk_t, ak2, kat2),
            ):
                pp = ps_mm.tile([128, S], F32, tag="mm", name="pp")
                for m in range(4):
                    nc.tensor.matmul(
                        pp[:], w_t[m][:], x_t[m][:], start=(m == 0), stop=(m == 3)
                    )
                pb = const.tile([128, S], F32, name="pb")
                nc.vector.tensor_scalar_add(pb[:], pp[:], b_t[:])
                pa = ps_mm.tile([128, S], F32, tag="mm", name="pa")
                nc.tensor.matmul(pa[:], a2[:], pb[:], start=True, stop=True)
                nc.vector.tensor_copy(outT[:], pa[:])

            # v [k, (h,hd)] as 4 tiles of [128, 128] (no bias; bv folded later)
            v_t = []
            for kc in range(4):
                pv = ps_mm.tile([128, S], F32, tag="mm", name="pv")
                for m in range(4):
                    nc.tensor.matmul(
                        pv[:, 0:128],
                        xv_t[m][:, kc * 128 : (kc + 1) * 128],
                        wv_t[m][:],
                        start=(m == 0),
                        stop=(m == 3),
                    )
                vt = const.tile([128, 128], F32, name=f"v_{kc}")
                nc.vector.tensor_copy(vt[:], pv[:, 0:128])
                v_t.append(vt)

            # attnT chunks: [k-chunk partitions, (group, query, head) columns]
            attnT = [
                const.tile([128, 2 * S], F32, name=f"attnT_{c}") for c in range(4)
            ]

            # ---------- main loop: scores + softmax + transpose ----------
            for g in range(G):
                sc_ps = ps_sc.tile([128, S], F32, tag="sc", name="sc_ps")
                for i in range(64):
                    q = g * 64 + i
                    tt = tpool.tile([128, S], RED_DT, tag="T", name="tt")
                    nc.scalar.activation(
                        tt[:], kat2[:], AF.Tanh, bias=qat2[:, q : q + 1]
                    )
                    nc.tensor.matmul(
                        sc_ps[:],
                        avb[:, 128 - 2 * i : 256 - 2 * i],
                        tt[:],
                        start=(i == 0),
                        stop=(i == 63),
                    )
                # softmax over the 512 free-dim entries of each (q, h) row
                mx = stats.tile([128, 1], F32, tag="mx", name="mx")
                nc.vector.tensor_reduce(
                    mx[:], sc_ps[:], axis=mybir.AxisListType.X,
                    op=mybir.AluOpType.max, negate=True,
                )
                esum = stats.tile([128, 1], F32, tag="esum", name="esum")
                attn_e = apool.tile([128, S], F32, tag="attn_e", name="attn_e")
                nc.scalar.activation(
                    attn_e[:], sc_ps[:], AF.Exp, bias=mx[:], accum_out=esum[:]
                )
                rec = stats.tile([128, 1], F32, tag="rec", name="rec")
                nc.vector.reciprocal(rec[:], esum[:])
                attn_n = apool.tile([128, S], F32, tag="attn_n", name="attn_n")
                nc.vector.tensor_scalar_mul(attn_n[:], attn_e[:], rec[:])
                nc.sync.dma_start(attn_out[g * 128 : (g + 1) * 128, :], attn_n[:])
                for c in range(4):
                    tp = ps_tp.tile([128, 128], F32, tag="tp", name="tp")
                    nc.tensor.transpose(
                        tp[:], attn_n[:, c * 128 : (c + 1) * 128], ident[:]
                    )
                    nc.vector.tensor_copy(
                        attnT[c][:, g * 128 : (g + 1) * 128], tp[:]
                    )

            # ---------- context: ctx^T[(h,hd), q] ----------
            ctx_ps = ps_mm.tile([128, S], F32, tag="mm", name="ctx_ps")
            for h in range(HP):
                for c in range(4):
                    rhs = attnT[c][:].rearrange(
                        "p (g i h) -> p g i h", g=G, i=64, h=HP
                    )[:, :, :, h]
                    nc.tensor.matmul(
                        ctx_ps[h * HD : (h + 1) * HD, :],
                        v_t[c][:, h * HD : (h + 1) * HD],
                        rhs,
                        start=(c == 0),
                        stop=(c == 3),
                        skip_group_check=True,
                    )
            ctxT = const.tile([128, S], F32, name="ctxT")
            nc.vector.tensor_scalar_add(ctxT[:], ctx_ps[:], bv_t[:])

            # ---------- output projection ----------
            for sc in range(4):
                op_ps = ps_mm.tile([128, S], F32, tag="mm", name="op_ps")
                nc.tensor.matmul(
                    op_ps[:], ctxT[:, sc * 128 : (sc + 1) * 128], wo_t[:],
                    start=True, stop=True,
                )
                ob = apool.tile([128, S], F32, tag="ob", name="ob")
                nc.vector.tensor_add(ob[:], op_ps[:], bo_rep[:])
                nc.sync.dma_start(out_part[sc * 128 : (sc + 1) * 128, :], ob[:])

    nc.compile()
    return nc


_NC_CACHE = None


def _get_nc():
    global _NC_CACHE
    if _NC_CACHE is None:
        _NC_CACHE = build_nc()
    return _NC_CACHE


def _prep_core_inputs(c, query, key_, value, Wq, bq, Wk, bk, Wv, bv, Wo, bo, Aq, Ak, av):
    b = c // 4
    hp = c % 4
    cols = slice(hp * 128, hp * 128 + 128)
    cc = np.ascontiguousarray
    return {
        "xqT": cc(query[b].T),
        "xkT": cc(key_[b].T),
        "xvT": cc(value[b].T),
        "wq2": cc(Wq[:, cols]),
        "wk2": cc(Wk[:, cols]),
        "wv2": cc(Wv[:, cols]),
        "wo2": cc(Wo[cols, :]),
        "bq2": cc(bq[cols][:, None]),
        "bk2": cc(bk[cols][:, None]),
        "bv2": cc(bv[cols][:, None]),
        "bo4": cc((bo * 0.25)[None, :]),
        "aq": cc(Aq),
        "ak": cc(Ak),
        "av2": cc(av[:, None]),
    }


def kernel(**inputs):
    f = lambda name: np.asarray(inputs[name], dtype=np.float32)
    args = (
        f("query"), f("key_"), f("value"),
        f("Wq"), f("bq"), f("Wk"), f("bk"), f("Wv"), f("bv"),
        f("Wo"), f("bo"), f("Aq"), f("Ak"), f("av"),
    )
    nc = _get_nc()
    in_maps = [_prep_core_inputs(c, *args) for c in range(NCORES)]
    res = run_bass_kernel_spmd(nc, in_maps, core_ids=list(range(NCORES)))
    results = res.results

    attn = np.empty((B, H, S, S), dtype=np.float32)
    out = np.zeros((B, S, D), dtype=np.float32)
    for c in range(NCORES):
        b = c // 4
        hp = c % 4
        a = results[c]["attn_out"]  # [1024, 512] rows = (g, i, h) interleaved
        a = a.reshape(G, 64, HP, S).transpose(2, 0, 1, 3).reshape(HP, S, S)
        attn[b, 2 * hp : 2 * hp + 2] = a
        out[b] += results[c]["out_part"]
    return out, attn


# revision 5
# speedup vs baseline: 1.1654x; 1.1557x over previous
"""Bass/Trainium2 kernel for additive (Bahdanau-style) multi-head attention.

Reference computation (B=2, S=512, D=512, H=8, HD=64):
    q = heads(query @ Wq + bq); k = heads(key_ @ Wk + bk); v = heads(value @ Wv + bv)
    scores[b,h,i,j] = sum_d tanh((q @ Aq)[b,h,i,d] + (k @ Ak)[b,h,j,d]) * av[d]
    attn = softmax(scores, -1); ctx = attn @ v; out = merge(ctx) @ Wo + bo
    returns (out, attn)

Sharding: 8 cores; core c handles batch b = c // 4 and head pair
h0 = 2*(c % 4), h0+1.  Each core computes its two heads' attention and a
partial output projection; the host sums the 4 partial outputs per batch.

Per-core device plan (v2 — ScalarE tanh streaming is the roofline):
  - Projections on TensorE (fp32, exact): qT2/kT2 [128=(h,hd), 512] then
    block-diag Aq/Ak matmul -> qAT2/kAT2 [128, 512(tokens)].
  - The [dd=(h,hd), q, k] pre-tanh sum qAT2[:,q] + kAT2[:,k] is built by two
    engines in parallel (ScalarE must not do it per-q; that serializes at
    (224+512) cycles per query):
      * DVE blocks (11 q each): one tensor_tensor add with stride-0 broadcast
        APs -> [128, 11*512] fp32, then one big ACTIVATE Tanh -> bf16.
      * PE blocks (2 q each): identity-matmul broadcast-adds accumulate
        qcol + k into PSUM [128, 2*512] (bf16 args), then one ACTIVATE Tanh
        reading PSUM -> bf16.
  - TensorE reduces over d with av (bf16): lhsT is a shifted view into a
    [128, 256] buffer whose columns 128/129 hold av for head0/head1 rows, so
    query q accumulates into PSUM rows (2*(q%64), 2*(q%64)+1) of a [128, 512]
    score bank; lhsT width is trimmed to 2*(q%64)+2 columns to cut LDWEIGHTS.
  - Softmax per row: DVE max (negated), ScalarE Exp(bias=-max, accum_out)
    reading PSUM, DVE reciprocal + per-partition multiply.
  - attn rows DMA out interleaved; host de-interleaves.
  - PE transpose of attn tiles -> attnT [k, (group,query,head)] for the
    context matmul; ctx^T [(h,hd), q] accumulated in PSUM (+bv per partition,
    exact because softmax rows sum to 1); output projection vs Wo row-slice
    (+bo/4 so the host-side sum of 4 partials reconstructs bo).
"""

import numpy as np

import concourse.bass as bass
import concourse.mybir as mybir
import concourse.tile as tile
from concourse import bacc
from concourse.bass_utils import run_bass_kernel_spmd
from concourse.masks import make_identity

F32 = mybir.dt.float32
BF16 = mybir.dt.bfloat16
RED_DT = BF16  # dtype of tanh output + av weights for the d-reduction matmul
AF = mybir.ActivationFunctionType

B, S, D, H = 2, 512, 512, 8
HD = D // H  # 64
HP = 2  # heads per core
NCORES = 8
G = S // 64  # score groups of 64 queries -> 8

DVE_BS = 11      # queries per DVE add-block
DVE_BLOCKS = 4   # DVE blocks per group (44 queries)
PE_BS = 2        # queries per PE add-block
PE_BLOCKS = 10   # PE blocks per group (20 queries)
assert DVE_BS * DVE_BLOCKS + PE_BS * PE_BLOCKS == 64


def build_nc():
    nc = bacc.Bacc("TRN2", target_bir_lowering=False, debug=False, num_devices=NCORES)

    # ---- DRAM I/O (per-core shards; same names on every core) ----
    xqT = nc.dram_tensor("xqT", [D, S], F32, kind="ExternalInput")
    xkT = nc.dram_tensor("xkT", [D, S], F32, kind="ExternalInput")
    xvT = nc.dram_tensor("xvT", [D, S], F32, kind="ExternalInput")
    wq2 = nc.dram_tensor("wq2", [D, 128], F32, kind="ExternalInput")
    wk2 = nc.dram_tensor("wk2", [D, 128], F32, kind="ExternalInput")
    wv2 = nc.dram_tensor("wv2", [D, 128], F32, kind="ExternalInput")
    wo2 = nc.dram_tensor("wo2", [128, D], F32, kind="ExternalInput")
    bq2 = nc.dram_tensor("bq2", [128, 1], F32, kind="ExternalInput")
    bk2 = nc.dram_tensor("bk2", [128, 1], F32, kind="ExternalInput")
    bv2 = nc.dram_tensor("bv2", [128, 1], F32, kind="ExternalInput")
    bo4 = nc.dram_tensor("bo4", [1, D], F32, kind="ExternalInput")
    aq = nc.dram_tensor("aq", [HD, HD], F32, kind="ExternalInput")
    ak = nc.dram_tensor("ak", [HD, HD], F32, kind="ExternalInput")
    av2 = nc.dram_tensor("av2", [HD, 1], F32, kind="ExternalInput")
    attn_out = nc.dram_tensor("attn_out", [2 * S, S], F32, kind="ExternalOutput")
    out_part = nc.dram_tensor("out_part", [S, D], F32, kind="ExternalOutput")

    with tile.TileContext(nc) as tc:
        with (
            tc.tile_pool(name="const", bufs=1) as const,
            tc.tile_pool(name="tpool", bufs=2) as tpool,
            tc.tile_pool(name="apool", bufs=2) as apool,
            tc.tile_pool(name="stats", bufs=4) as stats,
        ):
            # ---------- load inputs ----------
            xq_t = []
            xk_t = []
            xv_t = []
            for m in range(4):
                t = const.tile([128, S], F32, name=f"xq_{m}")
                nc.sync.dma_start(t[:], xqT[m * 128 : (m + 1) * 128, :])
                xq_t.append(t)
                t = const.tile([128, S], F32, name=f"xk_{m}")
                nc.sync.dma_start(t[:], xkT[m * 128 : (m + 1) * 128, :])
                xk_t.append(t)
                t = const.tile([128, S], F32, name=f"xv_{m}")
                nc.sync.dma_start(t[:], xvT[m * 128 : (m + 1) * 128, :])
                xv_t.append(t)
            wq_t = []
            wk_t = []
            wv_t = []
            for m in range(4):
                t = const.tile([128, 128], F32, name=f"wq_{m}")
                nc.sync.dma_start(t[:], wq2[m * 128 : (m + 1) * 128, :])
                wq_t.append(t)
                t = const.tile([128, 128], F32, name=f"wk_{m}")
                nc.sync.dma_start(t[:], wk2[m * 128 : (m + 1) * 128, :])
                wk_t.append(t)
                t = const.tile([128, 128], F32, name=f"wv_{m}")
                nc.sync.dma_start(t[:], wv2[m * 128 : (m + 1) * 128, :])
                wv_t.append(t)
            wo_t = const.tile([128, D], F32, name="wo_t")
            nc.sync.dma_start(wo_t[:], wo2[:, :])
            bq_t = const.tile([128, 1], F32, name="bq_t")
            nc.sync.dma_start(bq_t[:], bq2[:, :])
            bk_t = const.tile([128, 1], F32, name="bk_t")
            nc.sync.dma_start(bk_t[:], bk2[:, :])
            bv_t = const.tile([128, 1], F32, name="bv_t")
            nc.sync.dma_start(bv_t[:], bv2[:, :])
            # bo/4 replicated across partitions via stride-0 DMA
            bo_rep = const.tile([128, D], F32, name="bo_rep")
            bo_bcast = bass.AP(tensor=bo4.ap().tensor, offset=0, ap=[[0, 128], [1, D]])
            nc.sync.dma_start(bo_rep[:], bo_bcast)

            # block-diagonal Aq/Ak [128, 128]
            aq2 = const.tile([128, 128], F32, name="aq2")
            nc.vector.memset(aq2[:], 0.0)
            nc.sync.dma_start(aq2[0:HD, 0:HD], aq[:, :])
            nc.sync.dma_start(aq2[HD:128, HD:128], aq[:, :])
            ak2 = const.tile([128, 128], F32, name="ak2")
            nc.vector.memset(ak2[:], 0.0)
            nc.sync.dma_start(ak2[0:HD, 0:HD], ak[:, :])
            nc.sync.dma_start(ak2[HD:128, HD:128], ak[:, :])

            # shifted-av buffer: col 128 = av on head0 rows, col 129 on head1
            av_sb = const.tile([128, 1], F32, name="av_sb")
            nc.sync.dma_start(av_sb[0:HD, :], av2[:, :])
            nc.sync.dma_start(av_sb[HD:128, :], av2[:, :])
            avb = const.tile([128, 256], RED_DT, name="avb")
            nc.vector.memset(avb[:], 0.0)
            nc.vector.tensor_copy(avb[0:HD, 128:129], av_sb[0:HD, :])
            nc.vector.tensor_copy(avb[HD:128, 129:130], av_sb[HD:128, :])

            ident = const.tile([128, 128], F32, name="ident")
            make_identity(nc, ident[:])
            ident_b = const.tile([128, 128], BF16, name="ident_b")
            nc.vector.tensor_copy(ident_b[:], ident[:])

            # ---------- projections (own PSUM phase) ----------
            qat2 = const.tile([128, S], F32, name="qat2")
            kat2 = const.tile([128, S], F32, name="kat2")
            v_t = []
            with tc.tile_pool(name="ps_pro", bufs=2, space="PSUM") as ps_pro:
                for x_t, w_t, b_t, a2, outT in (
                    (xq_t, wq_t, bq_t, aq2, qat2),
                    (xk_t, wk_t, bk_t, ak2, kat2),
                ):
                    pp = ps_pro.tile([128, S], F32, tag="mm", name="pp")
                    for m in range(4):
                        nc.tensor.matmul(
                            pp[:], w_t[m][:], x_t[m][:], start=(m == 0), stop=(m == 3)
                        )
                    pb = const.tile([128, S], F32, name="pb")
                    nc.vector.tensor_scalar_add(pb[:], pp[:], b_t[:])
                    pa = ps_pro.tile([128, S], F32, tag="mm", name="pa")
                    nc.tensor.matmul(pa[:], a2[:], pb[:], start=True, stop=True)
                    nc.vector.tensor_copy(outT[:], pa[:])

                # v [k, (h,hd)] as 4 tiles of [128, 128]
                for kc in range(4):
                    pv = ps_pro.tile([128, S], F32, tag="mm", name="pv")
                    for m in range(4):
                        nc.tensor.matmul(
                            pv[:, 0:128],
                            xv_t[m][:, kc * 128 : (kc + 1) * 128],
                            wv_t[m][:],
                            start=(m == 0),
                            stop=(m == 3),
                        )
                    vt = const.tile([128, 128], F32, name=f"v_{kc}")
                    nc.vector.tensor_copy(vt[:], pv[:, 0:128])
                    v_t.append(vt)

            # bf16 copies of qAT2/kAT2 for the PE add path
            qb = const.tile([128, S], BF16, name="qb")
            nc.vector.tensor_copy(qb[:], qat2[:])
            kb = const.tile([128, S], BF16, name="kb")
            nc.vector.tensor_copy(kb[:], kat2[:])

            # attnT chunks: [k-chunk partitions, (group, query, head) columns]
            attnT = [
                const.tile([128, 2 * S], F32, name=f"attnT_{c}") for c in range(4)
            ]

            # ---------- main loop: scores + softmax + transpose ----------
            with (
                tc.tile_pool(name="ps_sc", bufs=2, space="PSUM") as ps_sc,
                tc.tile_pool(name="ps_p4", bufs=2, space="PSUM") as ps_p4,
                tc.tile_pool(name="ps_tp", bufs=2, space="PSUM") as ps_tp,
            ):
                for g in range(G):
                    sc_ps = ps_sc.tile([128, S], F32, tag="sc", name="sc_ps")

                    def emit_red(i, rhs):
                        if i == 0:
                            # full width: start=True must clear the whole bank
                            # (zero lhsT columns write zeros to rows 2..127)
                            nc.tensor.matmul(
                                sc_ps[:, :], avb[:, 128:256], rhs,
                                start=True, stop=False, skip_group_check=True,
                            )
                        else:
                            nc.tensor.matmul(
                                sc_ps[0 : 2 * i + 2, :],
                                avb[:, 128 - 2 * i : 130],
                                rhs,
                                start=False,
                                stop=(i == 63),
                                skip_group_check=True,
                            )

                    pending = None  # reductions for the previous block
                    # DVE add-blocks
                    for bI in range(DVE_BLOCKS):
                        q0 = g * 64 + bI * DVE_BS
                        tpre = tpool.tile([128, DVE_BS, S], F32, tag="tpre",
                                          name="tpre")
                        in0 = (
                            qat2[:, q0 : q0 + DVE_BS]
                            .unsqueeze(2)
                            .broadcast_to([128, DVE_BS, S])
                        )
                        in1 = kat2[:].unsqueeze(1).broadcast_to([128, DVE_BS, S])
                        nc.vector.tensor_add(tpre[:], in0, in1)
                        td = tpool.tile([128, DVE_BS, S], RED_DT, tag="td", name="td")
                        nc.scalar.activation(td[:], tpre[:], AF.Tanh)
                        if pending is not None:
                            pending()
                        i0 = bI * DVE_BS
                        pending = (
                            lambda td=td, i0=i0: [
                                emit_red(i0 + j, td[:, j, :]) for j in range(DVE_BS)
                            ]
                        )
                    # PE add-blocks
                    for pI in range(PE_BLOCKS):
                        i0 = DVE_BLOCKS * DVE_BS + pI * PE_BS
                        p4 = ps_p4.tile([128, PE_BS, S], F32, tag="p4", name="p4")
                        for j in range(PE_BS):
                            q = g * 64 + i0 + j
                            nc.tensor.matmul(
                                p4[:, j, :],
                                ident_b[:],
                                qb[:, q : q + 1].broadcast_to([128, S]),
                                start=True,
                                stop=False,
                                skip_group_check=True,
                            )
                            nc.tensor.matmul(
                                p4[:, j, :],
                                ident_b[:],
                                kb[:],
                                start=False,
                                stop=True,
                                skip_group_check=True,
                            )
                        t2 = tpool.tile([128, PE_BS, S], RED_DT, tag="t2", name="t2")
                        nc.scalar.activation(t2[:], p4[:], AF.Tanh)
                        if pending is not None:
                            pending()
                        pending = (
                            lambda t2=t2, i0=i0: [
                                emit_red(i0 + j, t2[:, j, :]) for j in range(PE_BS)
                            ]
                        )
                    pending()

                    # softmax over the 512 free-dim entries of each (q, h) row
                    mx = stats.tile([128, 1], F32, tag="mx", name="mx")
                    nc.vector.tensor_reduce(
                        mx[:], sc_ps[:], axis=mybir.AxisListType.X,
                        op=mybir.AluOpType.max, negate=True,
                    )
                    esum = stats.tile([128, 1], F32, tag="esum", name="esum")
                    attn_e = apool.tile([128, S], F32, tag="attn_e", name="attn_e")
                    nc.scalar.activation(
                        attn_e[:], sc_ps[:], AF.Exp, bias=mx[:], accum_out=esum[:]
                    )
                    rec = stats.tile([128, 1], F32, tag="rec", name="rec")
                    nc.vector.reciprocal(rec[:], esum[:])
                    attn_n = apool.tile([128, S], F32, tag="attn_n", name="attn_n")
                    nc.vector.tensor_scalar_mul(attn_n[:], attn_e[:], rec[:])
                    nc.sync.dma_start(attn_out[g * 128 : (g + 1) * 128, :], attn_n[:])
                    for c in range(4):
                        tp = ps_tp.tile([128, 128], F32, tag="tp", name="tp")
                        nc.tensor.transpose(
                            tp[:], attn_n[:, c * 128 : (c + 1) * 128], ident[:]
                        )
                        nc.vector.tensor_copy(
                            attnT[c][:, g * 128 : (g + 1) * 128], tp[:]
                        )

            # ---------- context + output projection (own PSUM phase) ----------
            with tc.tile_pool(name="ps_epi", bufs=2, space="PSUM") as ps_epi:
                ctx_ps = ps_epi.tile([128, S], F32, tag="mm", name="ctx_ps")
                for h in range(HP):
                    for c in range(4):
                        rhs = attnT[c][:].rearrange(
                            "p (g i h) -> p g i h", g=G, i=64, h=HP
                        )[:, :, :, h]
                        nc.tensor.matmul(
                            ctx_ps[h * HD : (h + 1) * HD, :],
                            v_t[c][:, h * HD : (h + 1) * HD],
                            rhs,
                            start=(c == 0),
                            stop=(c == 3),
                            skip_group_check=True,
                        )
                ctxT = const.tile([128, S], F32, name="ctxT")
                nc.vector.tensor_scalar_add(ctxT[:], ctx_ps[:], bv_t[:])

                for sc in range(4):
                    op_ps = ps_epi.tile([128, S], F32, tag="mm", name="op_ps")
                    nc.tensor.matmul(
                        op_ps[:], ctxT[:, sc * 128 : (sc + 1) * 128], wo_t[:],
                        start=True, stop=True,
                    )
                    ob = apool.tile([128, S], F32, tag="ob", name="ob")
                    nc.vector.tensor_add(ob[:], op_ps[:], bo_rep[:])
                    nc.sync.dma_start(out_part[sc * 128 : (sc + 1) * 128, :], ob[:])

    nc.compile()
    return nc


_NC_CACHE = None


def _get_nc():
    global _NC_CACHE
    if _NC_CACHE is None:
        _NC_CACHE = build_nc()
    return _NC_CACHE


def _prep_core_inputs(c, query, key_, value, Wq, bq, Wk, bk, Wv, bv, Wo, bo, Aq, Ak, av):
    b = c // 4
    hp = c % 4
    cols = slice(hp * 128, hp * 128 + 128)
    cc = np.ascontiguousarray
    return {
        "xqT": cc(query[b].T),
        "xkT": cc(key_[b].T),
        "xvT": cc(value[b].T),
        "wq2": cc(Wq[:, cols]),
        "wk2": cc(Wk[:, cols]),
        "wv2": cc(Wv[:, cols]),
        "wo2": cc(Wo[cols, :]),
        "bq2": cc(bq[cols][:, None]),
        "bk2": cc(bk[cols][:, None]),
        "bv2": cc(bv[cols][:, None]),
        "bo4": cc((bo * 0.25)[None, :]),
        "aq": cc(Aq),
        "ak": cc(Ak),
        "av2": cc(av[:, None]),
    }


def kernel(**inputs):
    f = lambda name: np.asarray(inputs[name], dtype=np.float32)
    args = (
        f("query"), f("key_"), f("value"),
        f("Wq"), f("bq"), f("Wk"), f("bk"), f("Wv"), f("bv"),
        f("Wo"), f("bo"), f("Aq"), f("Ak"), f("av"),
    )
    nc = _get_nc()
    in_maps = [_prep_core_inputs(c, *args) for c in range(NCORES)]
    res = run_bass_kernel_spmd(nc, in_maps, core_ids=list(range(NCORES)))
    results = res.results

    attn = np.empty((B, H, S, S), dtype=np.float32)
    out = np.zeros((B, S, D), dtype=np.float32)
    for c in range(NCORES):
        b = c // 4
        hp = c % 4
        a = results[c]["attn_out"]  # [1024, 512] rows = (g, i, h) interleaved
        a = a.reshape(G, 64, HP, S).transpose(2, 0, 1, 3).reshape(HP, S, S)
        attn[b, 2 * hp : 2 * hp + 2] = a
        out[b] += results[c]["out_part"]
    return out, attn
